# revision 15
# baseline (speedup 1.0000x reference)
"""Trainium2 kernel for nn_DeepLinearTimeSeries.

The reference network is a 400-layer *linear* residual MLP: every step is
x <- x @ (W_i^T) [+ 0.1 * carry], with no nonlinearities anywhere. The whole
stack therefore collapses algebraically to a single matrix:

    out = x @ M_total,   M_total = T_enc @ T_temp @ T_dec @ W_out^T  (64 x 1)

where each block's transfer matrix is the product of its per-layer factors
(W_i^T + 0.1*I), with the first two layers of the encoder/temporal blocks
handled per the reference's carry pattern (T = W0^T W1^T + 0.1 I).

We fold the 400 64x64 factors on the host (trivial FLOPs, same f32
arithmetic regime as the reference), then run the remaining memory-bound
pass y = x @ m on 8 NeuronCores, data-parallel over the batch dim
(sharding_hint). Per core: x shard [32768, 64] f32 (8 MiB) -> y [32768, 1].

Device mapping per core (hidden=64 is too small for tensor-engine
efficiency, and the collapsed op is a mat-vec, i.e. pure memory-bound):
rows are laid out contiguously per SBUF partition ("(p n) h -> p n h"),
the 64-vector m is broadcast across partitions host-side, and the vector
engine does a broadcast multiply + segmented reduce over the hidden dim.
"""

import numpy as np

import concourse.bass as bass
import concourse.mybir as mybir
import concourse.tile as tile
from concourse.bass_utils import run_bass_kernel_spmd

# Problem constants (hardcoded per harness contract).
B, S, H = 128, 2048, 64
N_CORES = 8
RW = np.float32(0.1)
ROWS = B * S // N_CORES          # 32768 rows per core
P = 128                          # SBUF partitions
NPP = ROWS // P                  # 256 rows per partition
CHUNK = 64                       # row-groups per DVE instruction
NCHUNK = NPP // CHUNK            # 4 chunks
FP32 = mybir.dt.float32

# Extra kwargs for run_bass_kernel_spmd (test harness sets these for tracing).
RUN_KWARGS: dict = {}


def _collapse_weights(W_enc, W_temp, W_dec, W_out):
    """Fold the full linear stack into a single [H, 1] f32 matrix."""
    eye = np.eye(H, dtype=np.float32)

    def block_mat(Ws):
        # x1 = x0 W0^T ; x2 = x1 W1^T + 0.1 x0 ; then x <- x (Wi^T + 0.1 I)
        T = Ws[0].T @ Ws[1].T + RW * eye
        for Wi in Ws[2:]:
            T = T @ (Wi.T + RW * eye)
        return T

    M = block_mat(W_enc) @ block_mat(W_temp)
    for Wd in W_dec:
        M = M @ (Wd.T + RW * eye)
    return (M @ W_out.T).astype(np.float32)  # [H, 1]


_drain_patched = False


def _patch_drain_waits():
    """Split the kernel-tail Drain's wait list into single-wait NOPs.

    The Drain's CTRL ISA struct holds fewer sync-wait slots than the number
    of distinct semaphore lanes a kernel can touch; walrus hard-errors on
    overflow. Semantically identical: SP blocks on each sem, then drains.
    """
    global _drain_patched
    if _drain_patched:
        return
    _drain_patched = True

    import ast

    from concourse.vector_clock import ScopedClock, VectorClock

    def _split_drain_and_barrier(self, tick_clock, wait_clock):
        # One NOP per active proc, each carrying at most one sem wait
        # (add_sem_waits updates the observed clock, so nothing repeats).
        rep = repr(tick_clock.global_clock)
        ticks = ast.literal_eval(rep[rep.index("(") + 1 : -1])
        for p, t in enumerate(ticks):
            if t <= 0:
                continue
            part = [0] * len(ticks)
            part[p] = t
            nop = self.nc.sync.nop()
            wait_clock.add_sem_waits(
                nop.ins, ScopedClock({None: VectorClock(part)})
            )
        self.nc.sync.drain()
        self.nc.all_engine_barrier()
        assert self.sems is not None
        popped = self.nc._tile_sem_poison_stack.pop()
        assert popped is self._sem_poison
        self.nc.clear_and_free_semaphores(list(self.sems.allocated().values()))
        self.nc.all_engine_barrier()

    tile.TileContext._drain_and_barrier = _split_drain_and_barrier


def _build_bass():
    # Tile round-robins DMA completion over 8 bookkeeping semaphore lanes
    # (DMAHW0-7). The kernel-tail Drain must wait on every lane used, and
    # its CTRL ISA struct can't hold that many waits. All our DMAs go
    # through the single SP HW-DGE ring (strict FIFO, and each InstDMACopy
    # already fans out across all 16 SDMA engines), so two lanes lose
    # nothing — and keep the Drain's wait list tiny.
    import concourse.tile_sem_assignment as _tsa

    _tsa.NUM_HWDGE_SEMS = 1
    _patch_drain_waits()

    nc = bass.Bass()
    x = nc.dram_tensor("x", [ROWS, H], FP32, kind="ExternalInput")
    m = nc.dram_tensor("m", [P, H], FP32, kind="ExternalInput")
    y = nc.dram_tensor("y", [ROWS, 1], FP32, kind="ExternalOutput")

    x_t = x.rearrange("(p n) h -> p n h", p=P)        # [128, 256, 64]
    y_t = y.rearrange("(p n) one -> p (n one)", p=P)  # [128, 256]

    # DVE compute instructions (TensorTensor etc.) have room for only ONE
    # sync-wait in their ISA struct. Discipline: every DVE op must need at
    # most one semaphore wait. Achieved by (a) bufs=NCHUNK pools so no slot
    # WAR waits exist, and (b) a tiny "toucher" copy per DMA'd tile that
    # absorbs the DMA-queue wait into the DVE engine's observed clock before
    # the real compute op consumes the tile.
    with tile.TileContext(nc) as tc:
        with (
            tc.tile_pool(name="xin", bufs=NCHUNK) as xpool,
            tc.tile_pool(name="prod", bufs=NCHUNK) as ppool,
            tc.tile_pool(name="scr", bufs=NCHUNK) as spool,
            tc.tile_pool(name="misc", bufs=1) as mpool,
        ):
            m_ld = mpool.tile([P, H], FP32, tag="m_ld")
            nc.sync.dma_start(m_ld[:], m[:])
            m_sb = mpool.tile([P, H], FP32, tag="m")
            nc.vector.tensor_copy(m_sb[:], m_ld[:])  # absorbs m's DMA wait
            acc = mpool.tile([P, NPP], FP32, tag="acc")

            m_b = m_sb[:].unsqueeze(1).broadcast_to((P, CHUNK, H))
            last_xt = None
            for c in range(NCHUNK):
                xt = xpool.tile([P, CHUNK * H], FP32)
                nc.sync.dma_start(
                    xt[:].rearrange("p (n h) -> p n h", h=H),
                    x_t[:, c * CHUNK : (c + 1) * CHUNK, :],
                )
                last_xt = xt
                scr = spool.tile([P, 1], FP32)
                nc.vector.tensor_copy(scr[:], xt[:, :1])  # absorbs x DMA wait
                prod = ppool.tile([P, CHUNK * H], FP32)
                nc.vector.tensor_mul(
                    prod[:].rearrange("p (n h) -> p n h", h=H),
                    xt[:].rearrange("p (n h) -> p n h", h=H),
                    m_b,
                )
                nc.vector.tensor_reduce(
                    acc[:, c * CHUNK : (c + 1) * CHUNK],
                    prod[:].rearrange("p (n h) -> p n h", h=H),
                    axis=mybir.AxisListType.X,
                    op=mybir.AluOpType.add,
                )
            # Output via SWDGE (gpsimd): it is the only DMA on the DMASW
            # lane, so it carries no lane-ordering wait — just the DVE data
            # wait. A sync-engine DMA here would need lane-order + DVE = 2
            # waits, which overflows the DMA ISA struct's single wait slot.
            nc.gpsimd.dma_start(y_t[:], acc[:])
    return nc


def kernel(**inputs: np.ndarray) -> np.ndarray:
    x = np.asarray(inputs["x"], dtype=np.float32)
    m = _collapse_weights(
        np.asarray(inputs["W_enc"], dtype=np.float32),
        np.asarray(inputs["W_temp"], dtype=np.float32),
        np.asarray(inputs["W_dec"], dtype=np.float32),
        np.asarray(inputs["W_out"], dtype=np.float32),
    )
    m_bcast = np.ascontiguousarray(np.broadcast_to(m.reshape(1, H), (P, H)))

    nc = _build_bass()
    shard_b = B // N_CORES
    in_maps = [
        {
            "x": np.ascontiguousarray(
                x[i * shard_b : (i + 1) * shard_b].reshape(ROWS, H)
            ),
            "m": m_bcast,
        }
        for i in range(N_CORES)
    ]
    res = run_bass_kernel_spmd(
        nc, in_maps, core_ids=list(range(N_CORES)), **RUN_KWARGS
    )
    return np.concatenate(
        [r["y"].reshape(shard_b, S, 1) for r in res.results], axis=0
    )


# revision 24
# speedup vs baseline: 1.1160x; 1.1160x over previous
"""Trainium2 kernel for nn_DeepLinearTimeSeries.

The reference network is a 400-layer *linear* residual MLP: every step is
x <- x @ (W_i^T) [+ 0.1 * carry], with no nonlinearities anywhere. The whole
stack therefore collapses algebraically to a single matrix:

    out = x @ M_total,   M_total = T_enc @ T_temp @ T_dec @ W_out^T  (64 x 1)

where each block's transfer matrix is the product of its per-layer factors
(W_i^T + 0.1*I), with the first two layers of the encoder/temporal blocks
handled per the reference's carry pattern (T = W0^T W1^T + 0.1 I).

We fold the 400 64x64 factors on the host (trivial FLOPs, same f32
arithmetic regime as the reference), then run the remaining memory-bound
pass y = x @ m on 8 NeuronCores, data-parallel over the batch dim
(sharding_hint). Per core: x shard [32768, 64] f32 (8 MiB) -> y [32768, 1].

Device mapping per core (hidden=64 is too small for tensor-engine
efficiency, and the collapsed op is a mat-vec, i.e. pure memory-bound):
rows are laid out contiguously per SBUF partition ("(p n) h -> p n h"),
the 64-vector m is broadcast across partitions host-side, and the vector
engine does a broadcast multiply + segmented reduce over the hidden dim.
"""

import numpy as np

import concourse.bass as bass
import concourse.mybir as mybir
import concourse.tile as tile
from concourse.bass_utils import run_bass_kernel_spmd

# Problem constants (hardcoded per harness contract).
B, S, H = 128, 2048, 64
N_CORES = 8
RW = np.float32(0.1)
ROWS = B * S // N_CORES          # 32768 rows per core
P = 128                          # SBUF partitions
NPP = ROWS // P                  # 256 rows per partition
CHUNK = 64                       # row-groups per DVE instruction
NCHUNK = NPP // CHUNK            # 4 chunks
FP32 = mybir.dt.float32

# Extra kwargs for run_bass_kernel_spmd (test harness sets these for tracing).
RUN_KWARGS: dict = {}


def _collapse_weights(W_enc, W_temp, W_dec, W_out):
    """Fold the full linear stack into a single [H, 1] f32 matrix."""
    eye = np.eye(H, dtype=np.float32)

    def block_mat(Ws):
        # x1 = x0 W0^T ; x2 = x1 W1^T + 0.1 x0 ; then x <- x (Wi^T + 0.1 I)
        T = Ws[0].T @ Ws[1].T + RW * eye
        for Wi in Ws[2:]:
            T = T @ (Wi.T + RW * eye)
        return T

    M = block_mat(W_enc) @ block_mat(W_temp)
    for Wd in W_dec:
        M = M @ (Wd.T + RW * eye)
    return (M @ W_out.T).astype(np.float32)  # [H, 1]


_drain_patched = False


def _patch_drain_waits():
    """Split the kernel-tail Drain's wait list into single-wait NOPs.

    The Drain's CTRL ISA struct holds fewer sync-wait slots than the number
    of distinct semaphore lanes a kernel can touch; walrus hard-errors on
    overflow. Semantically identical: SP blocks on each sem, then drains.
    """
    global _drain_patched
    if _drain_patched:
        return
    _drain_patched = True

    import ast

    from concourse.vector_clock import ScopedClock, VectorClock

    def _split_drain_and_barrier(self, tick_clock, wait_clock):
        # One NOP per active proc, each carrying at most one sem wait
        # (add_sem_waits updates the observed clock, so nothing repeats).
        rep = repr(tick_clock.global_clock)
        ticks = ast.literal_eval(rep[rep.index("(") + 1 : -1])
        for p, t in enumerate(ticks):
            if t <= 0:
                continue
            part = [0] * len(ticks)
            part[p] = t
            nop = self.nc.sync.nop()
            wait_clock.add_sem_waits(
                nop.ins, ScopedClock({None: VectorClock(part)})
            )
        self.nc.sync.drain()
        self.nc.all_engine_barrier()
        assert self.sems is not None
        popped = self.nc._tile_sem_poison_stack.pop()
        assert popped is self._sem_poison
        self.nc.clear_and_free_semaphores(list(self.sems.allocated().values()))
        self.nc.all_engine_barrier()

    tile.TileContext._drain_and_barrier = _split_drain_and_barrier


def _build_bass():
    # Tile round-robins DMA completion over 8 bookkeeping semaphore lanes
    # (DMAHW0-7). The kernel-tail Drain must wait on every lane used, and
    # its CTRL ISA struct can't hold that many waits. All our DMAs go
    # through the single SP HW-DGE ring (strict FIFO, and each InstDMACopy
    # already fans out across all 16 SDMA engines), so two lanes lose
    # nothing — and keep the Drain's wait list tiny.
    import concourse.tile_sem_assignment as _tsa

    # 4 completion lanes -> the four x-chunk DMAs run concurrently (with one
    # lane they serialize: each DMA carries a same-lane ordering wait on the
    # previous DMA's completion). The drain-splitter below handles the
    # resulting multi-lane drain waits.
    _tsa.NUM_HWDGE_SEMS = 4
    _patch_drain_waits()

    nc = bass.Bass()
    x = nc.dram_tensor("x", [ROWS, H], FP32, kind="ExternalInput")
    m = nc.dram_tensor("m", [P, H], FP32, kind="ExternalInput")
    y = nc.dram_tensor("y", [ROWS, 1], FP32, kind="ExternalOutput")

    x_t = x.rearrange("(p n) h -> p n h", p=P)        # [128, 256, 64]
    y_t = y.rearrange("(p n) one -> p (n one)", p=P)  # [128, 256]

    # DVE compute instructions (TensorTensor etc.) have room for only ONE
    # sync-wait in their ISA struct. Discipline: every DVE op must need at
    # most one semaphore wait. Achieved by (a) bufs=NCHUNK pools so no slot
    # WAR waits exist, and (b) a tiny "toucher" copy per DMA'd tile that
    # absorbs the DMA-queue wait into the DVE engine's observed clock before
    # the real compute op consumes the tile.
    with tile.TileContext(nc) as tc:
        with (
            tc.tile_pool(name="xin", bufs=NCHUNK) as xpool,
            tc.tile_pool(name="prod", bufs=NCHUNK) as ppool,
            tc.tile_pool(name="scr", bufs=NCHUNK) as spool,
            tc.tile_pool(name="misc", bufs=1) as mpool,
        ):
            m_ld = mpool.tile([P, H], FP32, tag="m_ld")
            nc.sync.dma_start(m_ld[:], m[:])
            # Materialize m repeated CHUNK times ([128, CHUNK*H]) by log
            # doubling: keeps the big multiply a flat 2D contiguous op (3D
            # APs cost ~35 cycles per inner-segment restart on DVE).
            m_rep = mpool.tile([P, CHUNK * H], FP32, tag="m_rep")
            nc.vector.tensor_copy(m_rep[:, :H], m_ld[:])  # absorbs m DMA wait
            w = H
            while w < CHUNK * H:
                nc.vector.tensor_copy(m_rep[:, w : 2 * w], m_rep[:, :w])
                w *= 2
            acc = mpool.tile([P, NPP], FP32, tag="acc")

            for c in range(NCHUNK):
                xt = xpool.tile([P, CHUNK * H], FP32)
                nc.sync.dma_start(
                    xt[:].rearrange("p (n h) -> p n h", h=H),
                    x_t[:, c * CHUNK : (c + 1) * CHUNK, :],
                )
                scr = spool.tile([P, 1], FP32)
                nc.vector.tensor_copy(scr[:], xt[:, :1])  # absorbs x DMA wait
                prod = ppool.tile([P, CHUNK * H], FP32)
                nc.vector.tensor_mul(prod[:], xt[:], m_rep[:])
                # Segmented sum via avg-pool over the hidden window; the
                # x64 window factor is pre-folded into m on the host.
                nc.vector.tensor_reduce(
                    acc[:, c * CHUNK : (c + 1) * CHUNK],
                    prod[:].rearrange("p (n h) -> p n h", h=H),
                    axis=mybir.AxisListType.X,
                    op=mybir.AluOpType.add,
                )
            # Output via SWDGE (gpsimd): it is the only DMA on the DMASW
            # lane, so it carries no lane-ordering wait — just the DVE data
            # wait. A sync-engine DMA here would need lane-order + DVE = 2
            # waits, which overflows the DMA ISA struct's single wait slot.
            nc.gpsimd.dma_start(y_t[:], acc[:])
    return nc


def kernel(**inputs: np.ndarray) -> np.ndarray:
    x = np.asarray(inputs["x"], dtype=np.float32)
    m = _collapse_weights(
        np.asarray(inputs["W_enc"], dtype=np.float32),
        np.asarray(inputs["W_temp"], dtype=np.float32),
        np.asarray(inputs["W_dec"], dtype=np.float32),
        np.asarray(inputs["W_out"], dtype=np.float32),
    )
    m_bcast = np.ascontiguousarray(np.broadcast_to(m.reshape(1, H), (P, H)))

    nc = _build_bass()
    shard_b = B // N_CORES
    in_maps = [
        {
            "x": np.ascontiguousarray(
                x[i * shard_b : (i + 1) * shard_b].reshape(ROWS, H)
            ),
            "m": m_bcast,
        }
        for i in range(N_CORES)
    ]
    res = run_bass_kernel_spmd(
        nc, in_maps, core_ids=list(range(N_CORES)), **RUN_KWARGS
    )
    return np.concatenate(
        [r["y"].reshape(shard_b, S, 1) for r in res.results], axis=0
    )


# revision 25
# speedup vs baseline: 1.1540x; 1.0341x over previous
"""Trainium2 kernel for nn_DeepLinearTimeSeries.

The reference network is a 400-layer *linear* residual MLP: every step is
x <- x @ (W_i^T) [+ 0.1 * carry], with no nonlinearities anywhere. The whole
stack therefore collapses algebraically to a single matrix:

    out = x @ M_total,   M_total = T_enc @ T_temp @ T_dec @ W_out^T  (64 x 1)

where each block's transfer matrix is the product of its per-layer factors
(W_i^T + 0.1*I), with the first two layers of the encoder/temporal blocks
handled per the reference's carry pattern (T = W0^T W1^T + 0.1 I).

We fold the 400 64x64 factors on the host (trivial FLOPs, same f32
arithmetic regime as the reference), then run the remaining memory-bound
pass y = x @ m on 8 NeuronCores, data-parallel over the batch dim
(sharding_hint). Per core: x shard [32768, 64] f32 (8 MiB) -> y [32768, 1].

Device mapping per core (hidden=64 is too small for tensor-engine
efficiency; the collapsed op is a mat-vec, i.e. memory-bound): rows are
laid out contiguously per SBUF partition ("(p n) h -> p n h"), and the
compute is y[p,n] = sum_h x[p,n,h]*m[h]: an elementwise multiply by a
repeated-m vector plus a segmented (window-64) reduce. The multiply pass
is split between the vector engine and gpsimd (the reduce can only run on
the vector engine), so the two data passes largely overlap.

Scheduling note: TRN2 compute/DMA ISA structs hold a single sync-wait
slot, so the kernel is built so that no instruction ever needs more than
one semaphore wait: per-engine "toucher" copies absorb DMA-queue waits
into each engine's observed clock, pools are sized so buffer slots are
never reused (no WAR waits), the output DMA goes through SWDGE on its own
lane, and the kernel-tail drain's wait list is split into single-wait
NOPs (_patch_drain_waits).
"""

import numpy as np

import concourse.bass as bass
import concourse.mybir as mybir
import concourse.tile as tile
from concourse.bass_utils import run_bass_kernel_spmd

# Problem constants (hardcoded per harness contract).
B, S, H = 128, 2048, 64
N_CORES = 8
RW = np.float32(0.1)
ROWS = B * S // N_CORES          # 32768 rows per core
P = 128                          # SBUF partitions
NPP = ROWS // P                  # 256 rows per partition
NCHUNK = 8
CHUNK = NPP // NCHUNK            # 32 row-groups per chunk
FREE = CHUNK * H                 # 2048 elements per partition per chunk
# Multiply-pass engine split: gpsimd takes these chunks, vector the rest.
# Vector also does every reduce; gpsimd multiplies are ~2x slower than
# vector, so 5/8 on gpsimd balances the two engines' total work.
POOL_CHUNKS = frozenset((1, 3, 4, 6, 7))
FP32 = mybir.dt.float32

# Extra kwargs for run_bass_kernel_spmd (test harness sets these for tracing).
RUN_KWARGS: dict = {}


def _collapse_weights(W_enc, W_temp, W_dec, W_out):
    """Fold the full linear stack into a single [H, 1] f32 matrix."""
    eye = np.eye(H, dtype=np.float32)

    def block_mat(Ws):
        # x1 = x0 W0^T ; x2 = x1 W1^T + 0.1 x0 ; then x <- x (Wi^T + 0.1 I)
        T = Ws[0].T @ Ws[1].T + RW * eye
        for Wi in Ws[2:]:
            T = T @ (Wi.T + RW * eye)
        return T

    M = block_mat(W_enc) @ block_mat(W_temp)
    for Wd in W_dec:
        M = M @ (Wd.T + RW * eye)
    return (M @ W_out.T).astype(np.float32)  # [H, 1]


_drain_patched = False


def _patch_drain_waits():
    """Split the kernel-tail Drain's wait list into single-wait NOPs.

    The Drain's CTRL ISA struct holds fewer sync-wait slots than the number
    of distinct semaphore lanes a kernel can touch; walrus hard-errors on
    overflow. Semantically identical: SP blocks on each sem, then drains.
    """
    global _drain_patched
    if _drain_patched:
        return
    _drain_patched = True

    import ast

    from concourse.vector_clock import ScopedClock, VectorClock

    def _split_drain_and_barrier(self, tick_clock, wait_clock):
        # One NOP per active proc, each carrying at most one sem wait
        # (add_sem_waits updates the observed clock, so nothing repeats).
        rep = repr(tick_clock.global_clock)
        ticks = ast.literal_eval(rep[rep.index("(") + 1 : -1])
        for p, t in enumerate(ticks):
            if t <= 0:
                continue
            part = [0] * len(ticks)
            part[p] = t
            nop = self.nc.sync.nop()
            wait_clock.add_sem_waits(
                nop.ins, ScopedClock({None: VectorClock(part)})
            )
        self.nc.sync.drain()
        self.nc.all_engine_barrier()
        assert self.sems is not None
        popped = self.nc._tile_sem_poison_stack.pop()
        assert popped is self._sem_poison
        self.nc.clear_and_free_semaphores(list(self.sems.allocated().values()))
        self.nc.all_engine_barrier()

    tile.TileContext._drain_and_barrier = _split_drain_and_barrier


def _build_m_rep(nc, engine, pool, m_ld, tag):
    """Materialize m repeated CHUNK times ([P, FREE]) on `engine` by log
    doubling, so each engine's multiplies read an engine-local operand
    (keeps every op's cross-engine waits at one)."""
    m_rep = pool.tile([P, FREE], FP32, tag=tag)
    engine.tensor_copy(m_rep[:, :H], m_ld[:])  # absorbs the m DMA wait
    w = H
    while w < FREE:
        engine.tensor_copy(m_rep[:, w : 2 * w], m_rep[:, :w])
        w *= 2
    return m_rep


def _build_bass():
    import concourse.tile_sem_assignment as _tsa

    _tsa.NUM_HWDGE_SEMS = 8  # input DMAs each get their own lane: no
    # lane-ordering stalls; the drain splitter handles the wait fan-in.
    _patch_drain_waits()

    nc = bass.Bass()
    x = nc.dram_tensor("x", [ROWS, H], FP32, kind="ExternalInput")
    m = nc.dram_tensor("m", [P, H], FP32, kind="ExternalInput")
    y = nc.dram_tensor("y", [ROWS, 1], FP32, kind="ExternalOutput")

    x_t = x.rearrange("(p n) h -> p n h", p=P)        # [128, 256, 64]
    y_t = y.rearrange("(p n) one -> p (n one)", p=P)  # [128, 256]

    with tile.TileContext(nc) as tc:
        with (
            tc.tile_pool(name="xin", bufs=NCHUNK) as xpool,
            tc.tile_pool(name="prod", bufs=NCHUNK) as ppool,
            tc.tile_pool(name="scr", bufs=NCHUNK) as spool,
            tc.tile_pool(name="misc", bufs=1) as mpool,
        ):
            m_ld = mpool.tile([P, H], FP32, tag="m_ld")
            nc.sync.dma_start(m_ld[:], m[:])
            m_rep_v = _build_m_rep(nc, nc.vector, mpool, m_ld, "m_rep_v")
            m_rep_p = _build_m_rep(nc, nc.gpsimd, mpool, m_ld, "m_rep_p")
            acc = mpool.tile([P, NPP], FP32, tag="acc")

            for c in range(NCHUNK):
                xt = xpool.tile([P, FREE], FP32)
                nc.sync.dma_start(
                    xt[:].rearrange("p (n h) -> p n h", h=H),
                    x_t[:, c * CHUNK : (c + 1) * CHUNK, :],
                )
                eng = nc.gpsimd if c in POOL_CHUNKS else nc.vector
                m_rep = m_rep_p if c in POOL_CHUNKS else m_rep_v
                scr = spool.tile([P, 1], FP32)
                eng.tensor_copy(scr[:], xt[:, :1])  # absorbs x DMA wait
                prod = ppool.tile([P, FREE], FP32)
                eng.tensor_mul(prod[:], xt[:], m_rep[:])
                nc.vector.tensor_reduce(
                    acc[:, c * CHUNK : (c + 1) * CHUNK],
                    prod[:].rearrange("p (n h) -> p n h", h=H),
                    axis=mybir.AxisListType.X,
                    op=mybir.AluOpType.add,
                )
            # Output via SWDGE (gpsimd): only DMA on its DMASW lane, so it
            # carries no lane-ordering wait — just the data wait. (A sync-
            # engine DMA here would need lane-order + data = 2 waits.)
            nc.gpsimd.dma_start(y_t[:], acc[:])
    return nc


def kernel(**inputs: np.ndarray) -> np.ndarray:
    x = np.asarray(inputs["x"], dtype=np.float32)
    m = _collapse_weights(
        np.asarray(inputs["W_enc"], dtype=np.float32),
        np.asarray(inputs["W_temp"], dtype=np.float32),
        np.asarray(inputs["W_dec"], dtype=np.float32),
        np.asarray(inputs["W_out"], dtype=np.float32),
    )
    m_bcast = np.ascontiguousarray(np.broadcast_to(m.reshape(1, H), (P, H)))

    nc = _build_bass()
    shard_b = B // N_CORES
    in_maps = [
        {
            "x": np.ascontiguousarray(
                x[i * shard_b : (i + 1) * shard_b].reshape(ROWS, H)
            ),
            "m": m_bcast,
        }
        for i in range(N_CORES)
    ]
    res = run_bass_kernel_spmd(
        nc, in_maps, core_ids=list(range(N_CORES)), **RUN_KWARGS
    )
    return np.concatenate(
        [r["y"].reshape(shard_b, S, 1) for r in res.results], axis=0
    )
